# revision 2
# baseline (speedup 1.0000x reference)
"""Trainium2 Bass kernel for nn_DistributionLossWithLabel — packed dual-view.

Reference computation (B=8192, C=64):
    lq = log(q); lp = log(p)
    positive[i] = mean_c p[i,c]*(lp[i,c]-lq[i,c])
    a[j]        = sum_c p[j,c]*lp[j,c] / C
    kl[i,j]     = a[j] - (lq @ p^T)[i,j] / C
    negative[i] = sum_j kl[i,j] + sum_j kl[i,j]*(1-L[i,j])
    loss        = sum_i positive[i]/negative[i]

Key reformulation (rows i sharded 8 ways):
    kl[i,j] = base[j] - dlq[i]·p[j]/C   (dlq = lq - mean_i lq, base >= 0)
    negative[i] = 2*(sum(base) - dlq[i]·psum/C) - R[i]
    R[i] = (L@base)[i] - dlq[i]·(L@p)[i]/C      <- the only L-dependent part

Dual-view packed labels: each byte encodes a PAIR of labels
(s = L[i,2t] + 2*L[i,2t+1]) as one of four bytes {0x00,0x38,0x40,0xC8}.
The byte triple satisfies det[[x_s,y_s,K_s]] = 0 where x/y are the
fp8e4m3/fp8e5m2 interpretations and K = L0+L1, which makes an exact
zero-bias linear decode possible: two DoubleRow passes over the SAME
bytes — pass A reads them as e4m3 with weights alpha = X00*v0+X01*v1,
pass B as e5m2 (AP bitcast) with weights beta = X10*v0+X11*v1 — and
both accumulate into one PSUM.  Residual decode noise is ~0.5 * |v0-v1|
per pair with random sign (sum of noise coefs forced to 0), averaging
out over 4096 pairs and 8192 rows.  Label DMA halves to 4MB/core; the
TensorEngine (64 DoubleRow matmuls @ ~216ns) becomes the bottleneck.

The base column is split into 3 fp8 levels (ratio 32) per view so the
pair-weight fp8 rounding of the dominant L@base term stays ~1e-5;
p-column rounding noise enters only through dlq (zero-mean over i).
"""

import sys

if "/opt/trn_rl_repo" not in sys.path:
    sys.path.insert(0, "/opt/trn_rl_repo")

import ml_dtypes
import numpy as np

import concourse.bass as bass
import concourse.tile as tile
from concourse import bacc, mybir

FP = mybir.dt.float32
BF = mybir.dt.bfloat16
F16 = mybir.dt.float16
F8 = mybir.dt.float8e4
F8E5 = mybir.dt.float8e5
ALU = mybir.AluOpType
PM = mybir.MatmulPerfMode

B_FULL = 8192
C = 64
N_CORES = 8
NAUG = 80        # 64 p cols + 3 base levels + 13 pad (16B-aligned chunk stride)
NROW = 67        # used output rows
SP_P = 4096.0    # p-column weight scale
SP_BH = 4096.0   # base level-0 scale
RL = 32.0        # base level ratio
NLEV = 3

# ---- dual-view decode constants ----
CODES = np.array([0x00, 0x38, 0x40, 0xC8], dtype=np.uint8)


def _decode_consts():
    xs = CODES.view(ml_dtypes.float8_e4m3).astype(np.float64)
    ys = CODES.view(ml_dtypes.float8_e5m2).astype(np.float64)
    A = np.stack([xs[1:], ys[1:]], axis=1)            # 3x2
    K3 = np.array([1.0, 1.0, 2.0])
    uw = np.linalg.lstsq(A, K3, rcond=None)[0]        # exact (det condition)
    assert np.abs(A @ uw - K3).max() < 1e-9
    L0vec = np.array([1.0, 0.0, 1.0])
    cvec = A.T @ np.ones(3)
    Kk = np.block([[A.T @ A, cvec[:, None]], [cvec[None, :], np.zeros((1, 1))]])
    sol = np.linalg.solve(Kk, np.concatenate([A.T @ L0vec, [2.0]]))
    X00, X10 = sol[:2]
    return X00, uw[0] - X00, X10, uw[1] - X10


X00, X01, X10, X11 = _decode_consts()


def build_nc(B=B_FULL, shard=B_FULL // N_CORES, debug=False):
    assert B % 512 == 0 and shard % 512 == 0
    T = B // 2                 # label pairs per row
    njc_p = T // 128           # 128-pair chunks
    ndch = njc_p // 2          # DoubleRow chunk pairs (16)
    WD = 2 * 2 * NAUG          # per-dch weight bytes/partition (A/B views x 2 subchunks)

    nc = bacc.Bacc("TRN2", target_bir_lowering=False, debug=debug)

    # head = [w(dch0), w(dch1) | pc0..pc3]: everything the first two dchs
    # need in one transfer (a 128-partition HWDGE transfer costs ~2.5us
    # +0.6us sem latency nearly independent of size, so one big head beats
    # several small ones)
    HEADW = 2 * WD + 4 * shard
    head_d = nc.dram_tensor("head", [128, HEADW], F8, kind="ExternalInput")
    wts_d = nc.dram_tensor("wts", [128, (ndch - 2) * WD], F8, kind="ExternalInput")
    lab_d = nc.dram_tensor(
        "labels", [128, (njc_p - 4) * shard], F8, kind="ExternalInput"
    )
    # dlqt padded to 128 partitions: the natural [67, shard] shape DMAs at
    # ~25GB/s (partition-sparse descriptors) and lands on the epilogue path
    dlqt_d = nc.dram_tensor("dlqt", [128, shard], F16, kind="ExternalInput")
    out_d = nc.dram_tensor("out", [1, shard], FP, kind="ExternalOutput")

    with tile.TileContext(nc) as tc:
        with (
            tc.tile_pool(name="const", bufs=1) as cp,
            tc.tile_pool(name="mps_ps", bufs=1, space="PSUM") as mps_ps,
            tc.tile_pool(name="acc_ps", bufs=1, space="PSUM") as acc_ps,
        ):
            head = cp.tile([128, HEADW], F8)
            wts = cp.tile([128, (ndch - 2) * WD], F8)
            LT = cp.tile([128, (njc_p - 4) * shard], F8)
            dlqt = cp.tile([128, shard], F16)
            ones1 = cp.tile([128, 1], BF)
            scr = cp.tile([128, 513], BF)

            # gpsimd finishes its preamble first (~6.2us); memsets here gate
            # the PE ramp-up dummies, so keep them off the busier engines
            nc.gpsimd.memset(ones1[:], 1.0)
            nc.gpsimd.memset(scr[:], 1.0)

            # ---------------- DMA schedule ----------------
            # dch d consumes label pchunks (2d, 2d+1) + weight slab d.
            # HWDGE (sync/scalar) throughput scales with per-partition
            # descriptor size (4KB -> ~115GB/s, 1KB -> ~60, 320B -> ~25),
            # so weights move in few partition-dense transfers and labels
            # in 4-pchunk (4KB/partition) tiles after a small ramp.
            # SWDGE (gpsimd) tolerates small descriptors (~125GB/s) but
            # starts ~1us late; it carries the bulk weights + mid tiles
            # + dlqt (which would otherwise crawl on the epilogue path).
            lab_ap = lab_d.ap()
            wts_ap = wts_d.ap()

            def w_dma(eng, d0, d1):  # weight slabs for dch [d0, d1), d>=2
                eng.dma_start(
                    out=wts[:, (d0 - 2) * WD : (d1 - 2) * WD],
                    in_=wts_ap[:, (d0 - 2) * WD : (d1 - 2) * WD],
                )

            def l_dma(eng, c0, n):  # label pchunks [c0, c0+n), c0>=4
                cs = slice((c0 - 4) * shard, (c0 - 4 + n) * shard)
                eng.dma_start(out=LT[:, cs], in_=lab_ap[:, cs])

            sy, sc, gp = nc.sync, nc.scalar, nc.gpsimd
            sy.dma_start(out=head[:], in_=head_d.ap())  # dch0,1 kit
            l_dma(sy, 8, 4)      # dch4,5
            l_dma(sy, 20, 4)     # dch10,11
            l_dma(sc, 4, 4)      # dch2,3 (solo-burst early)
            l_dma(sc, 12, 4)     # dch6,7
            w_dma(sc, 9, 16)
            l_dma(sc, 28, 4)     # dch14,15
            w_dma(gp, 2, 9)
            l_dma(gp, 16, 4)     # dch8,9
            l_dma(gp, 24, 4)     # dch12,13
            gp.dma_start(out=dlqt[:], in_=dlqt_d.ap())

            # ---------------- main loop ----------------
            headl = head[:, 2 * WD : HEADW].rearrange("p (n i) -> p n i", i=shard)
            headl5 = headl.bitcast(F8E5)
            headw = head[:, 0 : 2 * WD].rearrange("p (d x) -> p d x", x=WD)
            LTv = LT[:].rearrange("p (n i) -> p n i", i=shard)
            LTv5 = LTv.bitcast(F8E5)
            wv = wts[:].rearrange("p (d x) -> p d x", x=WD)
            # separate PSUM tiles per i-half so epilogue dependency tracking
            # lets half 0's reduction start before half 1's last matmul
            mps_h = [
                mps_ps.tile([128, 512], FP, name=f"mps_h{h}") for h in range(2)
            ]
            scr_ps = mps_ps.tile([128, 512], FP)

            # The TRN2 PE clock ramps with *continuous* use (1.2GHz until
            # ~3-6us of uninterrupted execution, then 2.4GHz) and any idle
            # resets it.  Dummy matmuls on scratch SBUF (no DMA deps) ramp
            # the clock while the first label tiles stream in; a dummy after
            # each early dch keeps the PE busy across marginal tile arrivals.
            N_PRE = 10
            N_INTER = {0: 2, 1: 1, 2: 1, 3: 1}

            def dummy_mm(n):
                for _ in range(n):
                    nc.tensor.matmul(
                        scr_ps[0:1, :], scr[:, 0:1], scr[:, 1:513],
                        start=True, stop=True,
                    )

            dummy_mm(N_PRE)
            for d in range(ndch):
                if d < 2:
                    wA = headw[:, d, 0 : 2 * NAUG]
                    wB = headw[:, d, 2 * NAUG : WD]
                else:
                    wA = wv[:, d - 2, 0 : 2 * NAUG]
                    wB = wv[:, d - 2, 2 * NAUG : WD]
                if d < 2:
                    mvA = headl[:, 2 * d : 2 * d + 2, :]
                    mvB = headl5[:, 2 * d : 2 * d + 2, :]
                else:
                    mvA = LTv[:, 2 * (d - 2) : 2 * (d - 2) + 2, :]
                    mvB = LTv5[:, 2 * (d - 2) : 2 * (d - 2) + 2, :]
                wA = wA.rearrange("p (n f) -> p n f", f=NAUG)
                wB = wB.rearrange("p (n f) -> p n f", f=NAUG)
                last = d == ndch - 1
                # last dch interleaves views per half so half 0 of PSUM
                # finalizes two matmuls early -> epilogue overlaps the tail
                order = ((0, 0), (1, 0), (0, 1), (1, 1)) if last else (
                    (0, 0), (0, 1), (1, 0), (1, 1))
                for v, h in order:
                    i0 = h * 512
                    nc.tensor.matmul(
                        mps_h[h][0:NAUG, :],
                        wA if v == 0 else wB,
                        (mvA if v == 0 else mvB)[:, :, i0 : i0 + 512],
                        start=(d == 0 and v == 0),
                        stop=(last and v == 1),
                        perf_mode=PM.DoubleRow,
                    )
                dummy_mm(N_INTER.get(d, 0))

            # ---------------- epilogue ----------------
            # acc[_, i] = sum_rows mps*dlqt = SP_BH * R[i]; host does the rest.
            PROD = cp.tile([128, shard], BF)
            acc = acc_ps.tile([128, shard], FP)
            out_sb = cp.tile([128, shard], FP)
            quar = shard // 4
            half = shard // 2
            for qd in range(4):
                qs = slice(qd * quar, (qd + 1) * quar)
                hs = slice((qd % 2) * quar, (qd % 2 + 1) * quar)
                nc.vector.tensor_tensor(
                    PROD[0:NROW, qs], mps_h[qd // 2][0:NROW, hs],
                    dlqt[0:NROW, qs], op=ALU.mult,
                )
                nc.tensor.matmul(
                    acc[0:1, qs], ones1[0:NROW, :], PROD[0:NROW, qs],
                    start=True, stop=True,
                )
                if qd == 1:
                    nc.scalar.copy(out_sb[0:1, 0:half], acc[0:1, 0:half])
                    nc.sync.dma_start(
                        out=out_d.ap()[:, 0:half], in_=out_sb[0:1, 0:half]
                    )
                if qd == 3:
                    nc.vector.tensor_copy(
                        out_sb[0:1, half:shard], acc[0:1, half:shard]
                    )
                    nc.scalar.dma_start(
                        out=out_d.ap()[:, half:shard], in_=out_sb[0:1, half:shard]
                    )

    nc.compile()
    return nc


_NC_CACHE = {}


def _get_nc(B, shard):
    key = (B, shard)
    if key not in _NC_CACHE:
        _NC_CACHE[key] = build_nc(B, shard)
    return _NC_CACHE[key]


def _f8(x):
    return np.asarray(x, np.float32).astype(ml_dtypes.float8_e4m3)


def _host_precompute(q, p, labels_matrix):
    """Shared (core-independent) device inputs + host-side terms."""
    B, nC = q.shape
    T = B // 2
    ndch = T // 256
    WD = 2 * 2 * NAUG

    q64 = q.astype(np.float64)
    p64 = p.astype(np.float64)
    lq = np.log(q64)
    lp = np.log(p64)
    pos = (p64 * (lp - lq)).mean(axis=1)            # [B]
    a = (p64 * lp).sum(axis=1) / nC
    lbar = lq.mean(axis=0)
    dlq = lq - lbar                                  # [B, C]
    base = a - p64 @ (lbar / nC)                     # [B]
    psum = p64.sum(axis=0)                           # [C]
    sum_base = base.sum()
    # host part of negative: 2*(sum_base - dlq.psum/C)
    neg_host = 2.0 * (sum_base - dlq @ (psum / nC))  # [B]

    # ---- pair weights ----
    P0, P1 = p64[0::2, :], p64[1::2, :]              # [T, C]
    WA = np.zeros((T, NAUG), dtype=ml_dtypes.float8_e4m3)
    WB = np.zeros((T, NAUG), dtype=ml_dtypes.float8_e4m3)
    WA[:, :nC] = _f8((X00 * P0 + X01 * P1) * SP_P)
    WB[:, :nC] = _f8((X10 * P0 + X11 * P1) * SP_P)
    b0, b1 = base[0::2], base[1::2]
    for W, Y0, Y1 in ((WA, X00, X01), (WB, X10, X11)):
        r = (Y0 * b0 + Y1 * b1) * SP_BH
        for k in range(NLEV):
            h8 = _f8(r * RL**k)
            W[:, nC + k] = h8
            r = r - h8.astype(np.float64) / RL**k
    # layout [128, ndch*WD]: pair t = d*256 + n*128 + pp; per-d slab is
    # [viewA(2,NAUG) | viewB(2,NAUG)]
    def lay(W):
        return W.reshape(ndch, 2, 128, NAUG).transpose(2, 0, 1, 3)  # [128,d,n,f]
    wts = np.stack([lay(WA), lay(WB)], axis=2)       # [128, d, v, n, f]
    wts = np.ascontiguousarray(wts.reshape(128, ndch * WD))

    # ---- packed labels (full, per-core sliced later) ----
    Lm = labels_matrix
    S = (Lm[:, 0::2] + 2.0 * Lm[:, 1::2]).astype(np.uint8)  # [B, T]
    bytes_full = CODES[S]                                   # [B(i), T]

    # ---- dlqt rows (padded to 128 partitions for DMA descriptor width) ----
    dlqt = np.zeros((128, B), dtype=np.float16)
    dlqt[:nC, :] = (-dlq * (SP_BH / SP_P / nC)).T
    for k in range(NLEV):
        dlqt[nC + k, :] = 1.0 / RL**k

    return dict(
        wts=wts, bytes_full=bytes_full, dlqt=dlqt,
        pos=pos, neg_host=neg_host,
    )


def make_in_maps(pre, B, n_cores=N_CORES):
    shard = B // n_cores
    T = B // 2
    njc_p = T // 128
    WD = 2 * 2 * NAUG
    wts_all = pre["wts"]                              # [128, ndch*WD] fp8
    wts_head = wts_all[:, 0 : 2 * WD]
    wts_rest = np.ascontiguousarray(wts_all[:, 2 * WD :])
    maps = []
    for k in range(n_cores):
        s = slice(k * shard, (k + 1) * shard)
        lab = bytes_full_to_core(pre["bytes_full"], s, njc_p, shard)
        head = np.concatenate([wts_head, lab[:, 0 : 4 * shard]], axis=1)
        maps.append(
            {
                "head": np.ascontiguousarray(head),
                "wts": wts_rest,
                "labels": np.ascontiguousarray(lab[:, 4 * shard :]),
                "dlqt": np.ascontiguousarray(pre["dlqt"][:, s]),
            }
        )
    return maps


def bytes_full_to_core(bytes_full, s, njc_p, shard):
    tr = bytes_full[s, :].T                           # [T, shard]
    lab = tr.reshape(njc_p, 128, shard).transpose(1, 0, 2)
    return np.ascontiguousarray(lab.reshape(128, njc_p * shard)).view(
        ml_dtypes.float8_e4m3
    )


def finish(pre, results, B, n_cores=N_CORES):
    shard = B // n_cores
    pos, neg_host = pre["pos"], pre["neg_host"]
    total = 0.0
    for k, r in enumerate(results):
        accv = r["out"].astype(np.float64).reshape(-1)
        s = slice(k * shard, (k + 1) * shard)
        neg = neg_host[s] - accv / SP_BH
        total += (pos[s] / neg).sum()
    return np.float32(total)


def kernel(q, p, labels_matrix):
    from concourse.bass_utils import run_bass_kernel_spmd

    q = np.asarray(q, dtype=np.float32)
    p = np.asarray(p, dtype=np.float32)
    labels_matrix = np.asarray(labels_matrix, dtype=np.float32)
    B = q.shape[0]
    shard = B // N_CORES
    nc = _get_nc(B, shard)
    pre = _host_precompute(q, p, labels_matrix)
    in_maps = make_in_maps(pre, B, N_CORES)
    res = run_bass_kernel_spmd(nc, in_maps, core_ids=list(range(N_CORES)))
    return finish(pre, res.results, B, N_CORES)
